# revision 1
# baseline (speedup 1.0000x reference)
"""Trainium2 Bass kernel for nn_MixedAttnHeadEmbed_82076825027210.

Computes, per batch element:
    out = sum over h in {4, 8, 12} of CausalAttention(Q_mix_h, K_mix_h, V_mix_h)
where Q/K/V_mix_h are weighted mixtures (9 scalar weights) of head-sliced
views of x's q/k/v channel groups, zero-padded per head to hd_max = 768/h.

Sharding: data-parallel over batch B=8 across the 8 NeuronCores (one batch
element per core); the 9 mixture weights are baked into the compiled program
as immediates.

Per-core plan (T=1024 tokens, bf16 compute, fp32 accumulation):
  1. SWDGE cast-DMA x [1024, 2304] f32 -> SBUF bf16 tiles.
  2. DVE builds the per-config mixed Q/K (natural layout) and V_aug
     (natural, with a ones-column per head for the softmax denominator).
  3. Q/K mixed naturals bounce through DRAM and return via HWDGE
     DMA-transpose as Q^T/K^T [d, T] (bf16), ready as matmul operands.
  4. For each config, in passes of 2-4 heads: S^T = K_mix Q_mix^T blockwise
     on PE (causal blocks only, diagonal masked via an extra matmul with a
     strict-triangular constant), exp on ACT (softmax scale folded in,
     max-subtraction skipped: |S*scale| << 1 so exp cannot overflow),
     then Y^T = V_aug^T P^T accumulated in PSUM over tk (the ones-column
     produces the softmax denominator l as an extra row of Y^T).
  5. Y^T -> SBUF (DVE), PE-transpose back to natural, normalize by 1/l and
     sum across configs with DVE scalar_tensor_tensor into the fp32 output
     accumulator; one DMA writes the result.
"""

import math

import numpy as np

import concourse.bass as bass
import concourse.bacc as bacc
import concourse.tile as tile
from concourse import mybir
from concourse.bass_utils import run_bass_kernel_spmd
from concourse.tile import add_dep_helper

F32 = mybir.dt.float32
BF16 = mybir.dt.bfloat16
ALU = mybir.AluOpType
ACTF = mybir.ActivationFunctionType

T = 1024
NT = 8  # token tiles of 128
E = 768
CIN = 3 * E
N_HEAD_LIST = (4, 8, 12)
EMBED_DIM_LIST = (384, 576, 768)
N_CORES = 8
MASK_NEG = -3000.0  # pre-scale additive mask; exp(scale*-3000) == 0 in f32


def _qtw(h):
    """Transposed-layout total rows: h=8 pads each 96-d head to 128 rows so
    every matmul operand slice starts at a legal base partition (0/32/64)."""
    return 1024 if h == 8 else E


def _dchunks(h):
    """Per head: contraction (d) ranges split at 128-row QT tile boundaries."""
    hd = E // h
    stride = _qtw(h) // h
    out = []
    for i in range(h):
        a, b = i * stride, i * stride + hd
        chunks = []
        while a < b:
            nxt = min(b, (a // 128 + 1) * 128)
            chunks.append((a, nxt))
            a = nxt
        out.append(chunks)
    return out


def _passes(h):
    if h == 4:
        return [[0, 1], [2, 3]]
    if h == 8:
        return [[0, 1, 2, 3], [4, 5, 6, 7]]
    return [[0, 1, 2, 3], [4, 5, 6, 7], [8, 9, 10, 11]]


def _vchunks(h):
    """Column ranges of one head's (hd+1)-wide V_aug block, <=128 rows each."""
    hd = E // h
    if hd + 1 > 128:
        return [(0, 128), (128, hd + 1)]
    return [(0, hd + 1)]


def _build_program(W):
    """W: numpy [9] f32 mixture weights. Returns compiled Bacc program."""
    nc = bacc.Bacc(
        "TRN2", target_bir_lowering=False, debug=False, num_devices=N_CORES
    )
    x_in = nc.dram_tensor("x", [T, CIN], F32, kind="ExternalInput").ap()
    out_d = nc.dram_tensor("out", [T, E], F32, kind="ExternalOutput").ap()
    qk_dram = [
        [
            nc.dram_tensor(
                f"qkb_{ci}_{ti}", [T, _qtw(N_HEAD_LIST[ci])], BF16
            ).ap()
            for ti in range(2)
        ]
        for ci in range(3)
    ]

    with tile.TileContext(nc) as tc:
        _emit(tc, x_in, out_d, qk_dram, W)
    nc.compile()
    return nc


def _emit(tc, x_in, out_d, qk_dram, W):
    nc = tc.nc
    with (
        tc.tile_pool(name="consts", bufs=1) as consts,
        tc.tile_pool(name="xin", bufs=4) as xpool,
        tc.tile_pool(name="qkm", bufs=6) as qkm_pool,
        tc.tile_pool(name="vtmp", bufs=2) as vtmp_pool,
        tc.tile_pool(name="vaug", bufs=1) as vaug_pool,
        tc.tile_pool(name="qt", bufs=4) as qt_pool,
        tc.tile_pool(name="pt", bufs=6) as pt_pool,
        tc.tile_pool(name="ytsb", bufs=6) as ytsb_pool,
        tc.tile_pool(name="small", bufs=8) as small_pool,
        tc.tile_pool(name="oacc", bufs=1) as oacc_pool,
        tc.tile_pool(name="stage", bufs=2, space="PSUM") as stage_pool,
        tc.tile_pool(name="ypsum", bufs=2, space="PSUM") as ypsum_pool,
    ):
        # ---- constants -------------------------------------------------
        ident = consts.tile([128, 128], BF16)
        nc.gpsimd.memset(ident, 0.0)
        nc.gpsimd.affine_select(
            out=ident, in_=ident, compare_op=ALU.not_equal, fill=1.0,
            base=0, pattern=[[-1, 128]], channel_multiplier=1,
        )
        # ustrict[d, t] = 1 if t > d else 0
        ustrict = consts.tile([128, 128], BF16)
        nc.gpsimd.memset(ustrict, 1.0)
        nc.gpsimd.affine_select(
            out=ustrict, in_=ustrict, compare_op=ALU.is_gt, fill=0.0,
            base=0, pattern=[[1, 128]], channel_multiplier=-1,
        )
        # negi = MASK_NEG * I
        negi = consts.tile([128, 128], BF16)
        nc.gpsimd.memset(negi, 0.0)
        nc.gpsimd.affine_select(
            out=negi, in_=negi, compare_op=ALU.not_equal, fill=MASK_NEG,
            base=0, pattern=[[-1, 128]], channel_multiplier=1,
        )

        oacc = oacc_pool.tile([128, NT, E], F32)

        # weight order in W: for cfg ci, e in (384, 576, 768): W[3*ci + idx]
        for ci, h in enumerate(N_HEAD_LIST):
            hd = E // h
            scale = 1.0 / math.sqrt(hd)
            dchunks = _dchunks(h)
            vchunks = _vchunks(h)
            pw = _qtw(h) // h

            # ---- mix this config (x re-loaded per cfg for overlap) ------
            vaug = vaug_pool.tile([128, NT, h, hd + 1], BF16, tag=f"vaug{ci}")
            nc.vector.memset(vaug[:, :, :, hd : hd + 1], 1.0)
            bounce = {0: [], 1: []}
            for t in range(NT):
                xt = xpool.tile([128, CIN], BF16, tag="x16")
                nc.gpsimd.dma_start(
                    out=xt[:, :], in_=x_in[t * 128 : (t + 1) * 128, :]
                )
                for tensor_idx in range(3):  # 0=Q 1=K 2=V
                    base = tensor_idx * E
                    if tensor_idx < 2:
                        eng = nc.vector
                        nat = qkm_pool.tile([128, h, pw], BF16, tag="qkm")
                        if pw > hd:
                            nc.vector.memset(nat[:, :, hd:pw], 0.0)

                        def out_ap(e):
                            return nat[:, :, 0 : e // h]
                    else:
                        eng = nc.gpsimd

                        def out_ap(e):
                            return vaug[:, t, :, 0 : e // h]

                    sl = xt[:, base : base + 768]
                    eng.tensor_scalar(
                        out_ap(768),
                        sl.rearrange("p (h d) -> p h d", h=h),
                        float(W[3 * ci + 2]),
                        None,
                        ALU.mult,
                    )
                    for e, wi in ((576, 1), (384, 0)):
                        sl = xt[:, base : base + e]
                        in0 = sl.rearrange("p (h d) -> p h d", h=h)
                        w = float(W[3 * ci + wi])
                        if tensor_idx < 2:
                            # DVE: fused multiply-accumulate
                            nc.vector.scalar_tensor_tensor(
                                out=out_ap(e), in0=in0, scalar=w,
                                in1=out_ap(e), op0=ALU.mult, op1=ALU.add,
                            )
                        else:
                            # Pool has no scalar_tensor_tensor: ts + tt pair
                            tmp = vtmp_pool.tile(
                                [128, h, 576 // h], BF16, tag="vtmp"
                            )
                            tv = tmp[:, :, 0 : e // h]
                            nc.gpsimd.tensor_scalar(tv, in0, w, None, ALU.mult)
                            nc.gpsimd.tensor_tensor(
                                out_ap(e), tv, out_ap(e), ALU.add
                            )
                    if tensor_idx < 2:
                        ins = nc.sync.dma_start(
                            out=qk_dram[ci][tensor_idx][
                                t * 128 : (t + 1) * 128, :
                            ],
                            in_=nat[:, :, :],
                        )
                        bounce[tensor_idx].append(ins)

            # ---- DMA-transpose Q/K back as [d, T] tiles -----------------
            ndt = _qtw(h) // 128
            qtk = []
            for ti in range(2):
                tl = qt_pool.tile([128, 8, T], BF16, tag="qt", name=f"qt{ci}{ti}")
                for half in range(2):
                    rows = slice(half * 512, half * 512 + 512)
                    for dt_ in range(ndt):
                        ins = nc.sync.dma_start(
                            out=tl[:, dt_, rows],
                            in_=qk_dram[ci][ti][
                                rows, dt_ * 128 : (dt_ + 1) * 128
                            ],
                            transpose=True,
                        )
                        for b in bounce[ti][half * 4 : half * 4 + 4]:
                            add_dep_helper(
                                ins.ins, b.ins, reason="dram bounce raw"
                            )
                qtk.append(tl)
            qt, kt = qtk

            # ---- attention for this config ------------------------------
            for pheads in _passes(h):
                nh = len(pheads)
                ncph = len(vchunks)
                # chunk tiles reordered so psum-pair partners have equal rows:
                # all heads' chunk 0, then all heads' chunk 1, ...
                ctiles = [
                    (hh, c0, c1) for (c0, c1) in vchunks for hh in pheads
                ]
                offs = []
                acc = 0
                for (_, c0, c1) in ctiles:
                    offs.append(acc)
                    acc += c1 - c0
                    acc = (acc + 1) & ~1  # psum bf16 needs 4B-aligned offsets
                # l column (within packed ynat cols) per head position
                lcols = [
                    offs[(ncph - 1) * nh + p] + (hd - vchunks[-1][0])
                    for p in range(nh)
                ]
                lbase = lcols[0]
                lstride = (lcols[1] - lcols[0]) if nh > 1 else 1
                assert all(
                    lcols[p] == lbase + p * lstride for p in range(nh)
                ), lcols
                groups = [pheads[i : i + 2] for i in range(0, nh, 2)]

                for s in range(2):
                    ntk = 4 * s + 4
                    pts = {}
                    yts = [
                        ypsum_pool.tile(
                            [128, 2, 512], F32, tag="y", name=f"yt{k}"
                        )
                        for k in range((len(ctiles) + 1) // 2)
                    ]
                    for tk in range(ntk):
                        lo = max(0, tk * 128 - s * 512)
                        for gi, g in enumerate(groups):
                            stage = stage_pool.tile([128, 2, 512], F32)
                            for j, head in enumerate(g):
                                n_mm = len(dchunks[head]) + (
                                    1 if tk // 4 == s else 0
                                )
                                mi = 0
                                for (a, b) in dchunks[head]:
                                    nc.tensor.matmul(
                                        out=stage[:, j, lo:512],
                                        lhsT=kt[
                                            a % 128 : a % 128 + (b - a),
                                            a // 128,
                                            tk * 128 : (tk + 1) * 128,
                                        ],
                                        rhs=qt[
                                            a % 128 : a % 128 + (b - a),
                                            a // 128,
                                            s * 512 + lo : (s + 1) * 512,
                                        ],
                                        start=(mi == 0),
                                        stop=(mi == n_mm - 1),
                                    )
                                    mi += 1
                                if tk // 4 == s:
                                    # diagonal block: MASK_NEG above diagonal
                                    nc.tensor.matmul(
                                        out=stage[:, j, lo : lo + 128],
                                        lhsT=ustrict[:, :],
                                        rhs=negi[:, :],
                                        start=False,
                                        stop=True,
                                    )
                            pt = pt_pool.tile([128, 2, 512], BF16, tag="pt")
                            nc.scalar.activation(
                                out=pt[:, 0 : len(g), lo:512],
                                in_=stage[:, 0 : len(g), lo:512],
                                func=ACTF.Exp,
                                scale=scale,
                            )
                            pts[gi] = pt
                        for k, (head, c0, c1) in enumerate(ctiles):
                            gi = pheads.index(head) // 2
                            j = pheads.index(head) % 2
                            nc.tensor.matmul(
                                out=yts[k // 2][0 : c1 - c0, k % 2, lo:512],
                                lhsT=vaug[:, tk, head, c0:c1],
                                rhs=pts[gi][:, j, lo:512],
                                start=(tk == 0),
                                stop=(tk == ntk - 1),
                            )
                    # ---- evict Y^T (ACT), transpose to natural ----------
                    ytsbs = []
                    for kp in range(len(yts)):
                        rows = ctiles[2 * kp][2] - ctiles[2 * kp][1]
                        nslots = min(2, len(ctiles) - 2 * kp)
                        sb = ytsb_pool.tile(
                            [128, 2, 512], BF16, tag="ytsb", name=f"ysb{kp}"
                        )
                        if kp % 2 == 0:
                            nc.scalar.copy(
                                sb[0:rows, 0:nslots, :],
                                yts[kp][0:rows, 0:nslots, :],
                            )
                        else:
                            nc.vector.tensor_copy(
                                sb[0:rows, 0:nslots, :],
                                yts[kp][0:rows, 0:nslots, :],
                            )
                        ytsbs.append(sb)
                    for tt in range(4):
                        tqg = s * 4 + tt
                        yn = ypsum_pool.tile(
                            [128, 512], BF16, tag="y", name="yn"
                        )
                        for k, (head, c0, c1) in enumerate(ctiles):
                            rows = c1 - c0
                            nc.tensor.transpose(
                                out=yn[:, offs[k] : offs[k] + rows],
                                in_=ytsbs[k // 2][
                                    0:rows, k % 2, tt * 128 : (tt + 1) * 128
                                ],
                                identity=ident[0:rows, 0:rows],
                            )
                        lrow = small_pool.tile([128, 4], F32, tag="lrow")
                        rec = small_pool.tile([128, 4], F32, tag="rec")
                        nc.vector.tensor_copy(
                            lrow[:, 0:nh],
                            yn[
                                :,
                                lbase : lbase + (nh - 1) * lstride + 1 : lstride,
                            ],
                        )
                        nc.vector.reciprocal(rec[:, 0:nh], lrow[:, 0:nh])
                        for k, (head, c0, c1) in enumerate(ctiles):
                            datarows = min(c1, hd) - c0
                            p = pheads.index(head)
                            dst = oacc[
                                :,
                                tqg,
                                head * hd + c0 : head * hd + c0 + datarows,
                            ]
                            src = yn[:, offs[k] : offs[k] + datarows]
                            if ci == 0:
                                nc.vector.tensor_scalar(
                                    dst, src, rec[:, p : p + 1], None, ALU.mult
                                )
                            else:
                                nc.vector.scalar_tensor_tensor(
                                    out=dst,
                                    in0=src,
                                    scalar=rec[:, p : p + 1],
                                    in1=dst,
                                    op0=ALU.mult,
                                    op1=ALU.add,
                                )

        # ---- write result ----------------------------------------------
        nc.sync.dma_start(
            out=out_d.rearrange("(a p) c -> p a c", p=128), in_=oacc[:, :, :]
        )


_PROGRAM_CACHE = {}


def _get_program(W):
    key = np.asarray(W, dtype=np.float32).tobytes()
    if key not in _PROGRAM_CACHE:
        _PROGRAM_CACHE[key] = _build_program(np.asarray(W, dtype=np.float32))
    return _PROGRAM_CACHE[key]


def kernel(x, weights):
    """x: [8, 1024, 2304] f32; weights: [9] f32 -> [8, 1024, 768] f32."""
    x = np.asarray(x, dtype=np.float32)
    weights = np.asarray(weights, dtype=np.float32)
    assert x.shape == (N_CORES, T, CIN), x.shape
    nc = _get_program(weights)
    in_maps = [{"x": np.ascontiguousarray(x[c])} for c in range(N_CORES)]
    res = run_bass_kernel_spmd(nc, in_maps, list(range(N_CORES)))
    return np.stack([res.results[c]["out"] for c in range(N_CORES)], axis=0)



# revision 7
# speedup vs baseline: 1489.4671x; 1489.4671x over previous
"""Trainium2 Bass kernel for nn_MixedAttnHeadEmbed_82076825027210.

Computes, per batch element:
    out = sum over h in {4, 8, 12} of CausalAttention(Q_mix_h, K_mix_h, V_mix_h)
where Q/K/V_mix_h are weighted mixtures (9 scalar weights) of head-sliced
views of x's q/k/v channel groups, padded per head to hd = 768/h.

Sharding: data-parallel over batch B=8 across the 8 NeuronCores (one batch
element per core); the 9 mixture weights are baked into the compiled program
as immediates.

Per-core plan (T=1024 tokens, bf16 compute, fp32 accumulation):
  1. One SWDGE cast-DMA loads x [1024, 2304] f32 -> SBUF bf16.
  2. Per config, DVE builds mixed Q/K naturals (and V_aug with a ones
     column per head for the softmax denominator) with fused
     scalar_tensor_tensor ops over whole-tensor access patterns.
  3. Q/K naturals bounce once through DRAM (SWDGE write) and return via
     HWDGE DMA-transpose as Q^T/K^T [d, T] bf16 matmul operands, with the
     transpose reads split across both HWDGE rings (SP + ACT).
  4. Attention per config, per 512-query block, per head-pass:
     S^T = K_mix Q_mix^T blockwise on PE (causal blocks only), exp on ACT
     (scale folded in; max-subtraction skipped since |S*scale| is small),
     diagonal-block causal mask applied post-exp via POOL affine_select,
     then Y = P V_aug accumulated *natural* (queries on partitions) in PSUM
     with P^T tiles as the stationary operand -- the ones-column lands the
     softmax denominator l as an extra output column.
  5. Per query tile: DVE reciprocal of l, then scalar_tensor_tensor
     normalize-and-accumulate from PSUM into the fp32 output accumulator;
     one DMA writes the result.
"""

import math

import numpy as np

import concourse.bass as bass
import concourse.bacc as bacc
import concourse.tile as tile
from concourse import mybir
from concourse.bass_utils import run_bass_kernel_spmd
from concourse.tile import add_dep_helper

F32 = mybir.dt.float32
BF16 = mybir.dt.bfloat16
ALU = mybir.AluOpType
ACTF = mybir.ActivationFunctionType

T = 1024
NT = 8  # token tiles of 128
E = 768
CIN = 3 * E
N_HEAD_LIST = (4, 8, 12)
EMBED_DIM_LIST = (384, 576, 768)
N_CORES = 8


def _pw(h):
    """Per-head column pitch in the natural mixed layout; h=8 pads 96 -> 128
    so every transposed head starts at a legal matmul base partition."""
    return 128 if h == 8 else E // h


def _dchunks(h):
    """Per head: contraction (d) row ranges in the transposed layout, split
    at 128-row tile boundaries."""
    hd = E // h
    pitch = _pw(h)
    out = []
    for i in range(h):
        a, b = i * pitch, i * pitch + hd
        chunks = []
        while a < b:
            nxt = min(b, (a // 128 + 1) * 128)
            chunks.append((a, nxt))
            a = nxt
        out.append(chunks)
    return out


def _passes(h):
    half = h // 2
    return [list(range(half)), list(range(half, h))]


def _build_program(W):
    """W: numpy [9] f32 mixture weights. Returns compiled Bacc program."""
    nc = bacc.Bacc(
        "TRN2", target_bir_lowering=False, debug=False, num_devices=N_CORES
    )
    x_in = nc.dram_tensor("x", [T, CIN], F32, kind="ExternalInput").ap()
    out_d = nc.dram_tensor("out", [T, E], F32, kind="ExternalOutput").ap()
    qk_dram = [
        [
            nc.dram_tensor(
                f"qkb_{ci}_{ti}", [T, N_HEAD_LIST[ci] * _pw(N_HEAD_LIST[ci])],
                BF16,
            ).ap()
            for ti in range(2)
        ]
        for ci in range(3)
    ]

    with tile.TileContext(nc) as tc:
        _emit(tc, x_in, out_d, qk_dram, W)
    nc.compile()
    return nc


def _emit(tc, x_in, out_d, qk_dram, W):
    nc = tc.nc
    with (
        tc.tile_pool(name="xbf", bufs=1) as xbf_pool,
        tc.tile_pool(name="nat", bufs=2) as nat_pool,
        tc.tile_pool(name="qkt", bufs=2) as qkt_pool,
        tc.tile_pool(name="vaug", bufs=2) as vaug_pool,
        tc.tile_pool(name="pt", bufs=4) as pt_pool,
        tc.tile_pool(name="small", bufs=4) as small_pool,
        tc.tile_pool(name="oacc", bufs=1) as oacc_pool,
        tc.tile_pool(name="stage", bufs=2, space="PSUM") as stage_pool,
        tc.tile_pool(name="ypsum", bufs=4, space="PSUM") as ypsum_pool,
    ):
        # ---- load x once, cast to bf16 ---------------------------------
        xbf = xbf_pool.tile([128, NT, CIN], BF16)
        nc.gpsimd.dma_start(
            out=xbf[:, :, :], in_=x_in.rearrange("(a p) c -> p a c", p=128)
        )

        oacc = oacc_pool.tile([128, NT, E], F32)

        # weight order in W: for cfg ci, e in (384, 576, 768): W[3*ci + idx]
        for ci, h in enumerate(N_HEAD_LIST):
            hd = E // h
            pw = _pw(h)
            scale = 1.0 / math.sqrt(hd)
            dchunks = _dchunks(h)
            ndt = h * pw // 128

            # ---- mix Q/K naturals (DVE), bounce, DMA-transpose back ----
            qkt = []
            bounce = []
            for tensor_idx in range(2):  # 0=Q 1=K
                base = tensor_idx * E
                nat = nat_pool.tile([128, NT, h, pw], BF16, tag="nat")
                if pw > hd:
                    nc.vector.memset(nat[:, :, :, hd:pw], 0.0)
                for k, e in ((2, 768), (1, 576), (0, 384)):
                    sl = xbf[:, :, base : base + e]
                    in0 = sl.rearrange("p a (h d) -> p a h d", h=h)
                    w = float(W[3 * ci + k])
                    if e == 768:
                        nc.vector.tensor_scalar(
                            nat[:, :, :, 0 : e // h], in0, w, None, ALU.mult
                        )
                    else:
                        nc.vector.scalar_tensor_tensor(
                            out=nat[:, :, :, 0 : e // h], in0=in0, scalar=w,
                            in1=nat[:, :, :, 0 : e // h],
                            op0=ALU.mult, op1=ALU.add,
                        )
                # write + transpose-reads of one tensor share one HWDGE
                # ring: per-ring FIFO makes the DRAM RAW ordering real
                # (add_dep_helper alone orders issue, not completion).
                eng = nc.sync
                wr = eng.dma_start(
                    out=qk_dram[ci][tensor_idx].rearrange(
                        "(a p) w -> p a w", p=128
                    ),
                    in_=nat[:, :, :, :],
                )
                bounce.append(wr)

                tl = qkt_pool.tile(
                    [128, ndt, T], BF16, tag="qkt", bufs=4,
                    name=f"qkt{ci}{tensor_idx}",
                )
                for dt_ in range(ndt):
                    rd = eng.dma_start(
                        out=tl[:, dt_, :],
                        in_=qk_dram[ci][tensor_idx][
                            :, dt_ * 128 : (dt_ + 1) * 128
                        ],
                        transpose=True,
                    )
                    add_dep_helper(rd.ins, wr.ins, reason="dram bounce raw")
                qkt.append(tl)
            qt, kt = qkt

            # ---- mix V_aug natural (DVE) -------------------------------
            vaug = vaug_pool.tile([128, NT, h, hd + 1], BF16, tag="vaug")
            nc.vector.memset(vaug[:, :, :, hd : hd + 1], 1.0)
            for k, e in ((2, 768), (1, 576), (0, 384)):
                sl = xbf[:, :, 2 * E : 2 * E + e]
                in0 = sl.rearrange("p a (h d) -> p a h d", h=h)
                w = float(W[3 * ci + k])
                if e == 768:
                    nc.vector.tensor_scalar(
                        vaug[:, :, :, 0 : e // h], in0, w, None, ALU.mult
                    )
                else:
                    nc.vector.scalar_tensor_tensor(
                        out=vaug[:, :, :, 0 : e // h], in0=in0, scalar=w,
                        in1=vaug[:, :, :, 0 : e // h],
                        op0=ALU.mult, op1=ALU.add,
                    )

            # ---- attention ---------------------------------------------
            for s in range(2):
                ntk = 4 * s + 4
                for pheads in _passes(h):
                    nh = len(pheads)
                    groups = [pheads[i : i + 2] for i in range(0, nh, 2)]
                    yts = [
                        ypsum_pool.tile(
                            [128, nh, hd + 1], F32, tag="y", name=f"yt{qt_}"
                        )
                        for qt_ in range(4)
                    ]
                    # One accumulation start per PSUM bank: start=True marks
                    # the whole 2KB zero region pending-zero, so only the
                    # first matmul in each Y bank may carry it; later heads'
                    # first writes overwrite via the pending-zero bytes.
                    y_first = [None] * 4
                    for tk in range(ntk):
                        lo = max(0, tk * 128 - s * 512)
                        for g in groups:
                            stage = stage_pool.tile(
                                [128, 2, 512], F32, tag="stage"
                            )
                            for j, head in enumerate(g):
                                chunks = dchunks[head]
                                for mi, (a, b) in enumerate(chunks):
                                    nc.tensor.matmul(
                                        out=stage[:, j, lo:512],
                                        lhsT=kt[
                                            a % 128 : a % 128 + (b - a),
                                            a // 128,
                                            tk * 128 : (tk + 1) * 128,
                                        ],
                                        rhs=qt[
                                            a % 128 : a % 128 + (b - a),
                                            a // 128,
                                            s * 512 + lo : (s + 1) * 512,
                                        ],
                                        start=(mi == 0),
                                        stop=(mi == len(chunks) - 1),
                                    )
                            ptl = pt_pool.tile([128, 2, 512], BF16, tag="pt")
                            nc.scalar.activation(
                                out=ptl[:, 0 : len(g), lo:512],
                                in_=stage[:, 0 : len(g), lo:512],
                                func=ACTF.Exp,
                                scale=scale,
                            )
                            if tk >= 4 * s:
                                # diagonal block: zero keys > query post-exp
                                dlo = tk * 128 - s * 512
                                for j in range(len(g)):
                                    nc.gpsimd.affine_select(
                                        out=ptl[:, j, dlo : dlo + 128],
                                        in_=ptl[:, j, dlo : dlo + 128],
                                        compare_op=ALU.is_ge,
                                        fill=0.0,
                                        base=0,
                                        pattern=[[1, 128]],
                                        channel_multiplier=-1,
                                    )
                            for qt_ in range(4):
                                qtg = 4 * s + qt_
                                if qtg < tk:
                                    continue
                                for j, head in enumerate(g):
                                    jp = pheads.index(head)
                                    is_start = (
                                        tk == 0 and y_first[qt_] is None
                                    )
                                    mm = nc.tensor.matmul(
                                        out=yts[qt_][:, jp, :],
                                        lhsT=ptl[
                                            :, j, qt_ * 128 : (qt_ + 1) * 128
                                        ],
                                        rhs=vaug[:, tk, head, :],
                                        start=is_start,
                                        stop=(tk == qtg and jp == nh - 1),
                                    )
                                    if is_start:
                                        y_first[qt_] = mm
                                    elif tk == 0:
                                        add_dep_helper(
                                            mm.ins,
                                            y_first[qt_].ins,
                                            reason="psum zero-region order",
                                        )
                    # ---- normalize + accumulate into oacc --------------
                    for qt_ in range(4):
                        tqg = 4 * s + qt_
                        lrow = small_pool.tile([128, 6], F32, tag="lrow")
                        rec = small_pool.tile([128, 6], F32, tag="rec")
                        nc.vector.tensor_copy(
                            lrow[:, 0:nh], yts[qt_][:, :, hd]
                        )
                        nc.vector.reciprocal(rec[:, 0:nh], lrow[:, 0:nh])
                        for jp, head in enumerate(pheads):
                            dst = oacc[
                                :, tqg, head * hd : head * hd + hd
                            ]
                            if ci == 0:
                                nc.vector.tensor_scalar(
                                    dst, yts[qt_][:, jp, 0:hd],
                                    rec[:, jp : jp + 1], None, ALU.mult,
                                )
                            else:
                                nc.vector.scalar_tensor_tensor(
                                    out=dst,
                                    in0=yts[qt_][:, jp, 0:hd],
                                    scalar=rec[:, jp : jp + 1],
                                    in1=dst,
                                    op0=ALU.mult,
                                    op1=ALU.add,
                                )

        # ---- write result ----------------------------------------------
        nc.sync.dma_start(
            out=out_d.rearrange("(a p) c -> p a c", p=128), in_=oacc[:, :, :]
        )


_PROGRAM_CACHE = {}


def _get_program(W):
    key = np.asarray(W, dtype=np.float32).tobytes()
    if key not in _PROGRAM_CACHE:
        _PROGRAM_CACHE[key] = _build_program(np.asarray(W, dtype=np.float32))
    return _PROGRAM_CACHE[key]


def kernel(x, weights):
    """x: [8, 1024, 2304] f32; weights: [9] f32 -> [8, 1024, 768] f32."""
    x = np.asarray(x, dtype=np.float32)
    weights = np.asarray(weights, dtype=np.float32)
    assert x.shape == (N_CORES, T, CIN), x.shape
    nc = _get_program(weights)
    in_maps = [{"x": np.ascontiguousarray(x[c])} for c in range(N_CORES)]
    res = run_bass_kernel_spmd(nc, in_maps, list(range(N_CORES)))
    return np.stack([res.results[c]["out"] for c in range(N_CORES)], axis=0)


# revision 11
# speedup vs baseline: 1522.0956x; 1.0219x over previous
"""Trainium2 Bass kernel for nn_MixedAttnHeadEmbed_82076825027210.

Computes, per batch element:
    out = sum over h in {4, 8, 12} of CausalAttention(Q_mix_h, K_mix_h, V_mix_h)
where Q/K/V_mix_h are weighted mixtures (9 scalar weights) of head-sliced
views of x's q/k/v channel groups, padded per head to hd = 768/h.

Sharding: data-parallel over batch B=8 across the 8 NeuronCores (one batch
element per core); the 9 mixture weights are baked into the compiled program
as immediates.

Per-core plan (T=1024 tokens, bf16 compute, fp32 accumulation):
  1. Six SWDGE cast-DMAs load x [1024, 2304] f32 -> SBUF bf16 in half-head
     column chunks so mixing starts as soon as the first chunk lands.
  2. Per config and per half of the heads, DVE builds mixed Q/K naturals
     (tensor_scalar at 4x + tensor_tensor adds at 2x -- scalar_tensor_tensor
     runs at 1x) and V_aug with a ones column per head for the softmax
     denominator.
  3. Each half bounces through DRAM and returns via HWDGE DMA-transpose as
     Q^T/K^T [d, T] bf16 matmul operands; Q uses the SP ring and K the ACT
     ring (per-ring FIFO makes the DRAM RAW ordering real), so the two
     tensors' DMAs overlap and attention pass 0 starts after half the work.
  4. Attention per config, per 512-query block, per half-of-heads pass:
     S^T = K_mix Q_mix^T blockwise on PE (causal blocks only; diagonal
     blocks masked by one extra ustrict x negi matmul per head), exp on ACT
     with the softmax scale folded in (max-subtraction skipped: |S*scale|
     is small), then Y = P V_aug accumulated *natural* (queries on
     partitions) in PSUM with P^T tiles as the stationary operand -- the
     ones-column lands the denominator l as an extra output column. PSUM
     start=True marks a whole 2KB zero region, so only the first matmul
     into each Y bank carries it (with explicit ordering deps).
  5. Per query tile: DVE reciprocal of l, then POOL scalar_tensor_tensor
     normalize-and-accumulate from PSUM into the fp32 output accumulator;
     the result streams out in per-query-block DMAs as configs finish.
"""

import math

import numpy as np

import concourse.bass as bass
import concourse.bacc as bacc
import concourse.tile as tile
from concourse import mybir
from concourse.bass_utils import run_bass_kernel_spmd
from concourse.tile import add_dep_helper

F32 = mybir.dt.float32
BF16 = mybir.dt.bfloat16
ALU = mybir.AluOpType
ACTF = mybir.ActivationFunctionType

T = 1024
NT = 8  # token tiles of 128
E = 768
CIN = 3 * E
N_HEAD_LIST = (4, 8, 12)
N_CORES = 8
MASK_NEG = -3000.0  # additive pre-scale mask; exp(scale*MASK_NEG) == 0


def _pw(h):
    """Per-head column pitch in the natural mixed layout; h=8 pads 96 -> 128
    so every transposed head starts at a legal matmul base partition."""
    return 128 if h == 8 else E // h


def _dchunks(h):
    """Per head: contraction (d) row ranges in the transposed layout, split
    at 128-row tile boundaries."""
    hd = E // h
    pitch = _pw(h)
    out = []
    for i in range(h):
        a, b = i * pitch, i * pitch + hd
        chunks = []
        while a < b:
            nxt = min(b, (a // 128 + 1) * 128)
            chunks.append((a, nxt))
            a = nxt
        out.append(chunks)
    return out


def _build_program(W):
    """W: numpy [9] f32 mixture weights. Returns compiled Bacc program."""
    nc = bacc.Bacc(
        "TRN2", target_bir_lowering=False, debug=False, num_devices=N_CORES
    )
    x_in = nc.dram_tensor("x", [T, CIN], F32, kind="ExternalInput").ap()
    out_d = nc.dram_tensor("out", [T, E], F32, kind="ExternalOutput").ap()
    qk_dram = [
        [
            nc.dram_tensor(
                f"qkb_{ci}_{ti}", [T, N_HEAD_LIST[ci] * _pw(N_HEAD_LIST[ci])],
                BF16,
            ).ap()
            for ti in range(2)
        ]
        for ci in range(3)
    ]

    with tile.TileContext(nc) as tc:
        _emit(tc, x_in, out_d, qk_dram, W)
    nc.compile()
    return nc


def _mix_half(nc, W, ci, out_ap, xsrc, tmp, e_list, h2):
    """Mixed half-tensor: out[:, :, i, 0:e/h] (+)= w_e * xsrc_e per e.
    tensor_scalar (4x) for the largest e, then ts into tmp + tensor_tensor
    add (2x) for the rest -- scalar_tensor_tensor would run at 1x."""
    # tmp is [128, NT, 288] flat; per half-config view [128, NT, h2, 288/h2]
    tview = tmp.rearrange("p a (h d) -> p a h d", h=h2)
    for idx, (k, e, hde) in enumerate(e_list):
        w = float(W[3 * ci + k])
        in0 = xsrc(e, hde)
        if idx == 0:
            nc.vector.tensor_scalar(
                out_ap(hde), in0, w, None, ALU.mult
            )
        else:
            tv = tview[:, :, :, 0:hde]
            nc.vector.tensor_scalar(tv, in0, w, None, ALU.mult)
            nc.vector.tensor_tensor(out_ap(hde), tv, out_ap(hde), ALU.add)


def _emit(tc, x_in, out_d, qk_dram, W):
    nc = tc.nc
    with (
        tc.tile_pool(name="consts", bufs=1) as consts,
        tc.tile_pool(name="xbf", bufs=1) as xbf_pool,
        tc.tile_pool(name="nat", bufs=2) as nat_pool,
        tc.tile_pool(name="tmp", bufs=1) as tmp_pool,
        tc.tile_pool(name="qkt", bufs=2) as qkt_pool,
        tc.tile_pool(name="vaug", bufs=2) as vaug_pool,
        tc.tile_pool(name="pt", bufs=4) as pt_pool,
        tc.tile_pool(name="small", bufs=4) as small_pool,
        tc.tile_pool(name="oacc", bufs=1) as oacc_pool,
        tc.tile_pool(name="stage", bufs=2, space="PSUM") as stage_pool,
        tc.tile_pool(name="ypsum", bufs=4, space="PSUM") as ypsum_pool,
    ):
        # ---- constants: strict-upper selector and MASK_NEG * I ----------
        ustrict = consts.tile([128, 128], BF16)
        nc.gpsimd.memset(ustrict, 1.0)
        nc.gpsimd.affine_select(
            out=ustrict, in_=ustrict, compare_op=ALU.is_gt, fill=0.0,
            base=0, pattern=[[1, 128]], channel_multiplier=-1,
        )
        negi = consts.tile([128, 128], BF16)
        nc.gpsimd.memset(negi, 0.0)
        nc.gpsimd.affine_select(
            out=negi, in_=negi, compare_op=ALU.not_equal, fill=MASK_NEG,
            base=0, pattern=[[-1, 128]], channel_multiplier=1,
        )

        # ---- load x in half-head column chunks, cast to bf16 ------------
        # order: Q half0, K half0, V half0, Q half1, K half1, V half1
        xbf = xbf_pool.tile([128, NT, CIN], BF16)
        for half in range(2):
            for third in range(3):
                c0 = third * E + half * (E // 2)
                nc.gpsimd.dma_start(
                    out=xbf[:, :, c0 : c0 + E // 2],
                    in_=x_in[:, c0 : c0 + E // 2].rearrange(
                        "(a p) c -> p a c", p=128
                    ),
                )

        oacc = oacc_pool.tile([128, NT, E], F32)

        # weight order in W: for cfg ci, e in (384, 576, 768): W[3*ci + idx]
        for ci, h in enumerate(N_HEAD_LIST):
            hd = E // h
            pw = _pw(h)
            h2 = h // 2
            scale = 1.0 / math.sqrt(hd)
            dchunks = _dchunks(h)
            ndt = h * pw // 128
            ndt2 = ndt // 2
            e_list = [(2, 768, hd), (1, 576, 576 // h), (0, 384, 384 // h)]

            # ---- mix + bounce + transpose per half of the heads ---------
            qkt = []
            vaug = vaug_pool.tile([128, NT, h, hd + 1], BF16, tag="vaug")
            for tensor_idx in range(2):
                tl = qkt_pool.tile(
                    [128, ndt, T], BF16, tag="qkt", bufs=4,
                    name=f"qkt{ci}{tensor_idx}",
                )
                qkt.append(tl)
            tmp = tmp_pool.tile([128, NT, 288], BF16, tag="tmp")
            for half in range(2):
                hsl = slice(half * h2, (half + 1) * h2)
                for tensor_idx in range(2):  # 0=Q (SP ring) 1=K (ACT ring)
                    base = tensor_idx * E
                    nat = nat_pool.tile(
                        [128, NT, h2, pw], BF16, tag="nat"
                    )
                    if pw > hd:
                        nc.vector.memset(nat[:, :, :, hd:pw], 0.0)

                    def xsrc(e, hde, base=base, half=half):
                        sl = xbf[
                            :, :,
                            base + half * (e // 2) : base + (half + 1) * (e // 2),
                        ]
                        return sl.rearrange("p a (h d) -> p a h d", h=h2)

                    def out_ap(hde, nat=nat):
                        return nat[:, :, :, 0:hde]

                    _mix_half(nc, W, ci, out_ap, xsrc, tmp, e_list, h2)

                    eng = nc.sync if tensor_idx == 0 else nc.scalar
                    w0 = half * h2 * pw
                    wr = eng.dma_start(
                        out=qk_dram[ci][tensor_idx][
                            :, w0 : w0 + h2 * pw
                        ].rearrange("(a p) w -> p a w", p=128),
                        in_=nat[:, :, :, :],
                    )
                    for dt_ in range(half * ndt2, (half + 1) * ndt2):
                        rd = eng.dma_start(
                            out=qkt[tensor_idx][:, dt_, :],
                            in_=qk_dram[ci][tensor_idx][
                                :, dt_ * 128 : (dt_ + 1) * 128
                            ],
                            transpose=True,
                        )
                        add_dep_helper(
                            rd.ins, wr.ins, reason="dram bounce raw"
                        )

                # V_aug for this half
                nc.vector.memset(vaug[:, :, hsl, hd : hd + 1], 1.0)

                def vsrc(e, hde, half=half):
                    sl = xbf[
                        :, :,
                        2 * E + half * (e // 2) : 2 * E + (half + 1) * (e // 2),
                    ]
                    return sl.rearrange("p a (h d) -> p a h d", h=h2)

                def vout(hde, hsl=hsl):
                    return vaug[:, :, hsl, 0:hde]

                _mix_half(nc, W, ci, vout, vsrc, tmp, e_list, h2)
            qt, kt = qkt

            # ---- attention ---------------------------------------------
            for s in range(2):
                ntk = 4 * s + 4
                for hf in range(2):
                    pheads = list(range(hf * h2, (hf + 1) * h2))
                    nh = h2
                    groups = [pheads[i : i + 2] for i in range(0, nh, 2)]
                    yts = [
                        ypsum_pool.tile(
                            [128, nh, hd + 1], F32, tag="y", name=f"yt{qt_}"
                        )
                        for qt_ in range(4)
                    ]
                    # One accumulation start per PSUM bank: start=True marks
                    # the whole 2KB zero region pending-zero, so only the
                    # first matmul in each Y bank carries it; later heads'
                    # first writes overwrite via the pending-zero bytes.
                    y_first = [None] * 4
                    for tk in range(ntk):
                        lo = max(0, tk * 128 - s * 512)
                        diag = tk >= 4 * s
                        dlo = tk * 128 - s * 512
                        for g in groups:
                            stage = stage_pool.tile(
                                [128, 2, 512], F32, tag="stage"
                            )
                            for j, head in enumerate(g):
                                chunks = dchunks[head]
                                n_mm = len(chunks) + (1 if diag else 0)
                                for mi, (a, b) in enumerate(chunks):
                                    nc.tensor.matmul(
                                        out=stage[:, j, lo:512],
                                        lhsT=kt[
                                            a % 128 : a % 128 + (b - a),
                                            a // 128,
                                            tk * 128 : (tk + 1) * 128,
                                        ],
                                        rhs=qt[
                                            a % 128 : a % 128 + (b - a),
                                            a // 128,
                                            s * 512 + lo : (s + 1) * 512,
                                        ],
                                        start=(mi == 0),
                                        stop=(mi == n_mm - 1),
                                    )
                                if diag:
                                    nc.tensor.matmul(
                                        out=stage[:, j, dlo : dlo + 128],
                                        lhsT=ustrict[:, :],
                                        rhs=negi[:, :],
                                        start=False,
                                        stop=True,
                                    )
                            ptl = pt_pool.tile([128, 2, 512], BF16, tag="pt")
                            nc.scalar.activation(
                                out=ptl[:, 0:2, lo:512],
                                in_=stage[:, 0:2, lo:512],
                                func=ACTF.Exp,
                                scale=scale,
                            )
                            for qt_ in range(4):
                                qtg = 4 * s + qt_
                                if qtg < tk:
                                    continue
                                for j, head in enumerate(g):
                                    jp = head - hf * h2
                                    is_start = (
                                        tk == 0 and y_first[qt_] is None
                                    )
                                    mm = nc.tensor.matmul(
                                        out=yts[qt_][:, jp, :],
                                        lhsT=ptl[
                                            :, j, qt_ * 128 : (qt_ + 1) * 128
                                        ],
                                        rhs=vaug[:, tk, head, :],
                                        start=is_start,
                                        stop=(tk == qtg and jp == nh - 1),
                                    )
                                    if is_start:
                                        y_first[qt_] = mm
                                    elif tk == 0:
                                        add_dep_helper(
                                            mm.ins,
                                            y_first[qt_].ins,
                                            reason="psum zero-region order",
                                        )
                    # ---- normalize + accumulate into oacc --------------
                    for qt_ in range(4):
                        tqg = 4 * s + qt_
                        lrow = small_pool.tile([128, 6], F32, tag="lrow")
                        rec = small_pool.tile([128, 6], F32, tag="rec")
                        nc.vector.tensor_copy(
                            lrow[:, 0:nh], yts[qt_][:, :, hd]
                        )
                        nc.vector.reciprocal(rec[:, 0:nh], lrow[:, 0:nh])
                        for jp, head in enumerate(pheads):
                            dst = oacc[:, tqg, head * hd : head * hd + hd]
                            if ci == 0:
                                nc.vector.tensor_scalar(
                                    dst, yts[qt_][:, jp, 0:hd],
                                    rec[:, jp : jp + 1], None, ALU.mult,
                                )
                            else:
                                nc.vector.scalar_tensor_tensor(
                                    out=dst,
                                    in0=yts[qt_][:, jp, 0:hd],
                                    scalar=rec[:, jp : jp + 1],
                                    in1=dst,
                                    op0=ALU.mult,
                                    op1=ALU.add,
                                )
                        if ci == 2 and hf == 1:
                            # rows for this query tile are final: stream out
                            nc.sync.dma_start(
                                out=out_d[
                                    tqg * 128 : (tqg + 1) * 128, :
                                ],
                                in_=oacc[:, tqg, :],
                            )


_PROGRAM_CACHE = {}


def _get_program(W):
    key = np.asarray(W, dtype=np.float32).tobytes()
    if key not in _PROGRAM_CACHE:
        _PROGRAM_CACHE[key] = _build_program(np.asarray(W, dtype=np.float32))
    return _PROGRAM_CACHE[key]


def kernel(x, weights):
    """x: [8, 1024, 2304] f32; weights: [9] f32 -> [8, 1024, 768] f32."""
    x = np.asarray(x, dtype=np.float32)
    weights = np.asarray(weights, dtype=np.float32)
    assert x.shape == (N_CORES, T, CIN), x.shape
    nc = _get_program(weights)
    in_maps = [{"x": np.ascontiguousarray(x[c])} for c in range(N_CORES)]
    res = run_bass_kernel_spmd(nc, in_maps, list(range(N_CORES)))
    return np.stack([res.results[c]["out"] for c in range(N_CORES)], axis=0)
